# revision 10
# baseline (speedup 1.0000x reference)
"""Trainium2 Bass kernel for a GPT-style transformer block.

B=4, T=2048, C=1024, H=16 heads (D=64), FF=4096.
Sharding: 8 NeuronCores, core c = 2*b + h handles batch b and a causally
*folded* pair of 512-token query chunks (h=0: global chunks {0,3}, h=1:
{1,2}), so every core does the same balanced amount of causal attention
work. K/V are computed on-core over the full sequence. One uniform SPMD
program; per-core causality enters only through data (host-permuted x and
DMA'd multiplicative masks).

All matmuls run in bf16 (PSUM accumulates fp32) — on this hardware f32r
lowers to fp32_mode=HIGH at less than half bf16 throughput, and the
error budget (2e-2 relative) has orders of magnitude of headroom.
Activations stay channel-major ([channel, token]) end to end: LN
statistics are taken over the partition dim with ones-matmuls, softmax
runs without max subtraction (scores are bounded), and V is augmented
with a ones column so the softmax denominator falls out of the AV matmul
itself.

Phase schedule is software-pipelined for the Tensor engine's DVFS ramp
(full clock only under sustained load): group-1 QKV matmuls are emitted
as filler between group-0 attention visits, and phase A's LN chunks
interleave with group-0 QKV.
"""
import sys

sys.path.insert(0, "/opt/trn_rl_repo")

import numpy as np
import ml_dtypes
from contextlib import ExitStack

import concourse.bass as bass
import concourse.tile as tile
from concourse import bacc, mybir

F32 = mybir.dt.float32
BF16 = mybir.dt.bfloat16
AF = mybir.ActivationFunctionType
OP = mybir.AluOpType

B, T, C, H, D = 4, 2048, 1024, 16, 64
FF = 4 * C
TOK = T // 2          # tokens owned per core (two 512-chunks: Y and X)
NCB = C // 128        # 8 channel blocks
NFB = FF // 128       # 32 ff blocks
NG = 2                # head groups
GH = H // NG          # 8 heads per group
NPAIR = GH // 2       # 4 head pairs per group
NSB = T // 128        # 16 key blocks
NMV = 16              # mask visits: 8 for Y (all) + 8 for X (last 8)

_CACHE = {}


def _build():
    nc = bacc.Bacc("TRN2", target_bir_lowering=False, debug=False, num_devices=8)

    xT_d = nc.dram_tensor("xT", [C, T], F32, kind="ExternalInput").ap()
    wqk_d = nc.dram_tensor("wqk", [NG, 2 * NPAIR, 128, NCB, 128], BF16,
                           kind="ExternalInput").ap()
    wv_d = nc.dram_tensor("wv", [NG, 128, NCB, 512], BF16, kind="ExternalInput").ap()
    wo_d = nc.dram_tensor("wo", [NCB, 128, NCB, 128], BF16, kind="ExternalInput").ap()
    wfc_d = nc.dram_tensor("wfc", [NFB, 128, NCB, 128], BF16, kind="ExternalInput").ap()
    wproj_d = nc.dram_tensor("wproj", [NCB, 128, NFB, 128], BF16,
                             kind="ExternalInput").ap()
    mask_d = nc.dram_tensor("masks", [128, NMV, 512], BF16, kind="ExternalInput").ap()
    out_d = nc.dram_tensor("outT", [C, TOK], F32, kind="ExternalOutput").ap()

    with tile.TileContext(nc) as tc, ExitStack() as top:
        persist = top.enter_context(tc.tile_pool(name="persist", bufs=1))
        ones_f = persist.tile([128, 128], F32)
        nc.vector.memset(ones_f[:], 1.0)
        ones_b = persist.tile([128, 128], BF16)
        nc.vector.tensor_copy(ones_b[:], ones_f[:])
        eps_t = persist.tile([128, 1], F32)
        nc.vector.memset(eps_t[:], 1e-5)
        y_sb = persist.tile([128, NCB, TOK], BF16)      # attention out (normalized)

        with tc.tile_pool(name="pLN", bufs=1) as pLN, \
             tc.tile_pool(name="pb", bufs=2) as pb, \
             tc.tile_pool(name="pbw", bufs=1) as pbw, \
             tc.tile_pool(name="psB", bufs=1, space="PSUM") as psB:
            lnr = pLN.tile([128, NCB, T], BF16)          # LN1(x), all tokens

            # ---------- Phase B: QKV for one group, as a fine generator ----
            def make_b(g):
                kT = pb.tile([128, NPAIR, T], BF16, tag="kT", name="kT")
                qT = pb.tile([128, NPAIR, TOK], BF16, tag="qT", name="qT")
                va = pb.tile([128, NSB, GH, 65], BF16, tag="vaug", name="va")

                def gen():
                    # preload K + V weights (Q weights streamed at the end)
                    wk = []
                    for p in range(NPAIR):
                        wtk = pbw.tile([128, NCB, 128], BF16, tag=f"wk{p}",
                                       name="wtk")
                        nc.sync.dma_start(wtk[:], wqk_d[g, NPAIR + p])
                        wk.append(wtk)
                    vw = pbw.tile([128, NCB, 512], BF16, tag="vw", name="vw")
                    nc.sync.dma_start(vw[:], wv_d[g])
                    for sblk in range(NSB):
                        nc.vector.tensor_copy(
                            va[:, sblk, :, 64:65],
                            ones_f[:, 0:1].broadcast_to([128, GH, 1]))
                    yield
                    # K and V, chunk-major so they pipeline with phase A
                    for n in range(4):
                        sl = slice(n * 512, (n + 1) * 512)
                        for p in range(NPAIR):
                            acc = psB.tile([128, 512], F32, tag="qk", name="acc")
                            for cb in range(NCB):
                                nc.tensor.matmul(acc[:], wk[p][:, cb, :],
                                                 lnr[:, cb, sl],
                                                 start=(cb == 0),
                                                 stop=(cb == NCB - 1))
                                if cb == 3:
                                    yield
                            nc.vector.tensor_copy(kT[:, p, sl], acc[:])
                            yield
                        for si in range(4):
                            sblk = n * 4 + si
                            vps = psB.tile([128, 512], F32, tag="vps", name="vps")
                            for cb in range(NCB):
                                nc.tensor.matmul(
                                    vps[:],
                                    lnr[:, cb, sblk * 128:(sblk + 1) * 128],
                                    vw[:, cb, :],
                                    start=(cb == 0), stop=(cb == NCB - 1))
                                if cb == 3:
                                    yield
                            nc.vector.tensor_copy(va[:, sblk, :, 0:64], vps[:])
                            yield
                    # Q for the two owned 512-chunks (stream Q weights)
                    for p in range(NPAIR):
                        wtq = pbw.tile([128, NCB, 128], BF16, tag="wq",
                                       name="wtq", bufs=2)
                        nc.sync.dma_start(wtq[:], wqk_d[g, p])
                        for ni, sl in enumerate((slice(0, 512),
                                                 slice(1536, 2048))):
                            acc = psB.tile([128, 512], F32, tag="qk", name="acc")
                            for cb in range(NCB):
                                nc.tensor.matmul(acc[:], wtq[:, cb, :],
                                                 lnr[:, cb, sl],
                                                 start=(cb == 0),
                                                 stop=(cb == NCB - 1))
                                if cb == 3:
                                    yield
                            nc.vector.tensor_copy(
                                qT[:, p, ni * 512:(ni + 1) * 512], acc[:])
                            yield

                return kT, qT, va, gen()

            def drain(gen_, k=None):
                if gen_ is None:
                    return
                try:
                    if k is None:
                        for _ in gen_:
                            pass
                    else:
                        for _ in range(k):
                            next(gen_)
                except StopIteration:
                    pass

            kT0, qT0, va0, bgen0 = make_b(0)

            # ---------- Phase A: LN1 in 256-token chunks, f32 stats ----------
            with tc.tile_pool(name="pax", bufs=2) as pax, \
                 tc.tile_pool(name="paw", bufs=3) as paw, \
                 tc.tile_pool(name="psA", bufs=2, space="PSUM") as psA:

                def a_unit(tck):
                    sl = slice(tck * 256, (tck + 1) * 256)
                    sum_ps = psA.tile([1, 256], F32, tag="sum", name="sum_ps")
                    sq_ps = psA.tile([1, 256], F32, tag="sq", name="sq_ps")
                    xfs = []
                    for cb in range(NCB):
                        xf = pax.tile([128, 256], F32, tag=f"xf{cb}", name="xf")
                        nc.sync.dma_start(xf[:], xT_d[cb * 128:(cb + 1) * 128, sl])
                        x2 = paw.tile([128, 256], F32, tag="x2", name="x2")
                        nc.vector.tensor_tensor(x2[:], xf[:], xf[:], OP.mult)
                        nc.tensor.matmul(sum_ps[:], ones_f[:, 0:1], xf[:],
                                         start=(cb == 0), stop=(cb == NCB - 1))
                        nc.tensor.matmul(sq_ps[:], ones_f[:, 0:1], x2[:],
                                         start=(cb == 0), stop=(cb == NCB - 1))
                        xfs.append(xf)
                    mean_r = paw.tile([1, 256], F32, tag="mean", name="mean_r")
                    var_r = paw.tile([1, 256], F32, tag="var", name="var_r")
                    nc.scalar.mul(mean_r[:], sum_ps[:], 1.0 / C)
                    nc.scalar.mul(var_r[:], sq_ps[:], 1.0 / C)
                    msq = paw.tile([1, 256], F32, tag="msq", name="msq")
                    nc.vector.tensor_mul(msq[:], mean_r[:], mean_r[:])
                    nc.vector.tensor_sub(var_r[:], var_r[:], msq[:])
                    nc.scalar.activation(var_r[:], var_r[:], AF.Sqrt,
                                         bias=eps_t[0:1, :])
                    nc.vector.reciprocal_approx_fast(out=var_r[:], in_=var_r[:])
                    mb = paw.tile([128, 256], F32, tag="mb", name="mb")
                    rb = paw.tile([128, 256], F32, tag="rb", name="rb")
                    nc.gpsimd.partition_broadcast(mb[:], mean_r[:])
                    nc.gpsimd.partition_broadcast(rb[:], var_r[:])
                    for cb in range(NCB):
                        xc = paw.tile([128, 256], F32, tag="xc", name="xc")
                        nc.vector.tensor_sub(xc[:], xfs[cb][:], mb[:])
                        nc.vector.tensor_tensor(lnr[:, cb, sl], xc[:], rb[:],
                                                OP.mult)

                # interleave A (8 units) with B0 K/V per 512-chunk
                drain(bgen0, 1)           # weight DMAs + ones columns
                a_unit(0)
                a_unit(1)
                a_unit(2)
                a_unit(3)
                drain(bgen0, 17)          # K/V chunk 0
                a_unit(4)
                drain(bgen0, 8)
                a_unit(5)
                drain(bgen0, 9)           # K/V chunk 1
                a_unit(6)
                drain(bgen0, 8)
                a_unit(7)
                drain(bgen0)              # rest of K/V + Q

            # ---------- Phase C ----------
            with tc.tile_pool(name="pcm", bufs=1) as pcm, \
                 tc.tile_pool(name="pct", bufs=3) as pct, \
                 tc.tile_pool(name="pcn", bufs=2) as pcn, \
                 tc.tile_pool(name="psS", bufs=2, space="PSUM") as psS, \
                 tc.tile_pool(name="psY", bufs=1, space="PSUM") as psY:
                masks = pcm.tile([128, NMV, 512], BF16)
                nc.sync.dma_start(masks[:], mask_d)

                def c_phase(g, kT, qT, va, filler):
                    for chunk in (1, 0):   # X first (12 unmasked visits lead)
                        trip = 8 if chunk == 0 else 16
                        mask_lo = 0 if chunk == 0 else 8
                        qsl = slice(chunk * 512, (chunk + 1) * 512)
                        for pair in range(NPAIR):
                            y01 = psY.tile([65, 1024], F32, tag="y01",
                                           name="y01")
                            for j in range(trip):
                                st = (j == 0)
                                sp = (j == trip - 1)
                                jsl = slice(j * 128, (j + 1) * 128)
                                s01 = psS.tile([128, 1024], F32, tag="s01",
                                               name="s01")
                                nc.tensor.matmul(s01[:, 0:512],
                                                 kT[0:64, pair, jsl],
                                                 qT[0:64, pair, qsl],
                                                 start=True, stop=True,
                                                 tile_position=(0, 0))
                                nc.tensor.matmul(s01[:, 512:1024],
                                                 kT[64:128, pair, jsl],
                                                 qT[64:128, pair, qsl],
                                                 start=True, stop=True,
                                                 tile_position=(64, 0))
                                p01r = pct.tile([128, 1024], BF16, tag="p01r",
                                                name="p01r")
                                if j >= mask_lo:
                                    p01 = pct.tile([128, 1024], BF16,
                                                   tag="p01", name="p01")
                                    nc.scalar.activation(p01[:], s01[:],
                                                         AF.Exp, scale=0.125)
                                    m2 = masks[:, j:j + 1, :].broadcast_to(
                                        [128, 2, 512])
                                    nc.vector.tensor_tensor(p01r[:], p01[:],
                                                            m2, OP.mult)
                                else:
                                    nc.scalar.activation(p01r[:], s01[:],
                                                         AF.Exp, scale=0.125)
                                drain(filler, 1)
                                nc.tensor.matmul(y01[:, 0:512],
                                                 va[:, j, 2 * pair, :],
                                                 p01r[:, 0:512], start=st,
                                                 stop=sp)
                                nc.tensor.matmul(y01[:, 512:1024],
                                                 va[:, j, 2 * pair + 1, :],
                                                 p01r[:, 512:1024], start=st,
                                                 stop=sp)
                            # normalize: y /= denom (ones-column of v_aug)
                            ysb = pcn.tile([65, 1024], F32, tag="ysb",
                                           name="ysb")
                            nc.vector.tensor_copy(ysb[:], y01[:])
                            den0 = pcn.tile([1, 1024], F32, tag="den0",
                                            name="den0")
                            nc.sync.dma_start(den0[:], ysb[64:65, :])
                            nc.vector.reciprocal_approx_fast(out=den0[:],
                                                             in_=den0[:])
                            rb01 = pcn.tile([64, 1024], F32, tag="rb01",
                                            name="rb01")
                            nc.gpsimd.partition_broadcast(rb01[:], den0[:],
                                                          channels=64)
                            blk = g * NPAIR + pair
                            yn1 = pcn.tile([64, 512], BF16, tag="yn1",
                                           name="yn1")
                            nc.vector.tensor_tensor(y_sb[0:64, blk, qsl],
                                                    ysb[0:64, 0:512],
                                                    rb01[:, 0:512], OP.mult)
                            nc.vector.tensor_tensor(yn1[:],
                                                    ysb[0:64, 512:1024],
                                                    rb01[:, 512:1024], OP.mult)
                            nc.sync.dma_start(y_sb[64:128, blk, qsl], yn1[:])

                kT1, qT1, va1, bgen1 = make_b(1)
                drain(bgen1, 1)
                c_phase(0, kT0, qT0, va0, bgen1)
                drain(bgen1)
                c_phase(1, kT1, qT1, va1, None)

        # ============ Phase D: out-projection + residual + LN2 ============
        with tc.tile_pool(name="pd", bufs=1) as pd:
            x1 = pd.tile([128, NCB, TOK], F32)           # x + attn
            ln2r = pd.tile([128, NCB, TOK], BF16)
            with tc.tile_pool(name="pdw", bufs=2) as pdw, \
                 tc.tile_pool(name="pdt", bufs=2) as pdt, \
                 tc.tile_pool(name="psD", bufs=2, space="PSUM") as psD:
                for ocb in range(NCB):
                    wt = pdw.tile([128, NCB, 128], BF16, tag="wt")
                    nc.sync.dma_start(wt[:], wo_d[ocb])
                    acc = psD.tile([128, 1024], F32, tag="proj")
                    for cblk in range(NCB):
                        for n2 in range(2):
                            nc.tensor.matmul(acc[:, n2 * 512:(n2 + 1) * 512],
                                             wt[:, cblk, :],
                                             y_sb[:, cblk, n2 * 512:(n2 + 1) * 512],
                                             start=(cblk == 0), stop=(cblk == NCB - 1))
                    xf = pdt.tile([128, TOK], F32, tag="xres")
                    # own tokens: rotated cols [0:512) (Y) and [1536:2048) (X)
                    nc.sync.dma_start(xf[:, 0:512],
                                      xT_d[ocb * 128:(ocb + 1) * 128, 0:512])
                    nc.sync.dma_start(xf[:, 512:1024],
                                      xT_d[ocb * 128:(ocb + 1) * 128, 1536:2048])
                    nc.vector.tensor_add(x1[:, ocb, :], acc[:], xf[:])

            # LN2 stats + normalize
            with tc.tile_pool(name="pet", bufs=3) as pet, \
                 tc.tile_pool(name="psE", bufs=2, space="PSUM") as psE:
                for tcks in range(TOK // 512):
                    sl = slice(tcks * 512, (tcks + 1) * 512)
                    sum_ps = psE.tile([1, 512], F32, tag="sum")
                    sq_ps = psE.tile([1, 512], F32, tag="sq")
                    for cb in range(NCB):
                        xr = pet.tile([128, 512], BF16, tag="xr")
                        nc.scalar.copy(xr[:], x1[:, cb, sl])
                        x2 = pet.tile([128, 512], BF16, tag="x2")
                        nc.vector.tensor_tensor(x2[:], xr[:], xr[:], OP.mult)
                        nc.tensor.matmul(sum_ps[:], ones_b[:, 0:1], xr[:],
                                         start=(cb == 0), stop=(cb == NCB - 1))
                        nc.tensor.matmul(sq_ps[:], ones_b[:, 0:1], x2[:],
                                         start=(cb == 0), stop=(cb == NCB - 1))
                    mean_r = pet.tile([1, 512], F32, tag="mean")
                    var_r = pet.tile([1, 512], F32, tag="var")
                    nc.scalar.mul(mean_r[:], sum_ps[:], 1.0 / C)
                    nc.scalar.mul(var_r[:], sq_ps[:], 1.0 / C)
                    msq = pet.tile([1, 512], F32, tag="msq")
                    nc.vector.tensor_mul(msq[:], mean_r[:], mean_r[:])
                    nc.vector.tensor_sub(var_r[:], var_r[:], msq[:])
                    nc.scalar.activation(var_r[:], var_r[:], AF.Sqrt,
                                         bias=eps_t[0:1, :])
                    nc.vector.reciprocal_approx_fast(out=var_r[:], in_=var_r[:])
                    mb = pet.tile([128, 512], F32, tag="mb")
                    rb = pet.tile([128, 512], F32, tag="rb")
                    nc.gpsimd.partition_broadcast(mb[:], mean_r[:])
                    nc.gpsimd.partition_broadcast(rb[:], var_r[:])
                    for cb in range(NCB):
                        xc = pet.tile([128, 512], F32, tag="xc")
                        nc.vector.tensor_sub(xc[:], x1[:, cb, sl], mb[:])
                        nc.vector.tensor_tensor(ln2r[:, cb, sl], xc[:], rb[:], OP.mult)

            # ============ Phase E: MLP (single pass over all own tokens) ====
            with tc.tile_pool(name="ph", bufs=1) as ph, \
                 tc.tile_pool(name="pew", bufs=3) as pew, \
                 tc.tile_pool(name="pepw", bufs=2) as pepw, \
                 tc.tile_pool(name="peo", bufs=3) as peo, \
                 tc.tile_pool(name="psF", bufs=2, space="PSUM") as psF:
                h_r = ph.tile([128, NFB, TOK], BF16)
                for fb in range(NFB):
                    wt = pew.tile([128, NCB, 128], BF16, tag="fwt")
                    nc.sync.dma_start(wt[:], wfc_d[fb])
                    fc = psF.tile([128, 1024], F32, tag="fc")
                    for cb in range(NCB):
                        for n2 in range(2):
                            nc.tensor.matmul(fc[:, n2 * 512:(n2 + 1) * 512],
                                             wt[:, cb, :],
                                             ln2r[:, cb, n2 * 512:(n2 + 1) * 512],
                                             start=(cb == 0), stop=(cb == NCB - 1))
                    nc.scalar.activation(h_r[:, fb, :], fc[:], AF.Gelu)
                for ocb in range(NCB):
                    wt = pepw.tile([128, NFB, 128], BF16, tag="pwt")
                    nc.sync.dma_start(wt[:], wproj_d[ocb])
                    acc = psF.tile([128, 1024], F32, tag="pacc")
                    for fb in range(NFB):
                        for n2 in range(2):
                            nc.tensor.matmul(acc[:, n2 * 512:(n2 + 1) * 512],
                                             wt[:, fb, :],
                                             h_r[:, fb, n2 * 512:(n2 + 1) * 512],
                                             start=(fb == 0), stop=(fb == NFB - 1))
                    of = peo.tile([128, TOK], F32, tag="of")
                    nc.vector.tensor_add(of[:], acc[:], x1[:, ocb, :])
                    nc.sync.dma_start(out_d[ocb * 128:(ocb + 1) * 128, :], of[:])

    nc.compile()
    return nc


def _prep_weights(g1, w_qkv, w_o, g2, w_fc, w_proj):
    g1 = np.asarray(g1, np.float32)
    g2 = np.asarray(g2, np.float32)
    wqkvT = np.ascontiguousarray((np.asarray(w_qkv, np.float32) * g1[None, :]).T)
    woT = np.ascontiguousarray(np.asarray(w_o, np.float32).T)
    wfcT = np.ascontiguousarray((np.asarray(w_fc, np.float32) * g2[None, :]).T)
    wprojT = np.ascontiguousarray(np.asarray(w_proj, np.float32).T)

    bf = ml_dtypes.bfloat16
    # wqk[g, ocb, r, cb, f] = wqkvT[cb*128 + r, col0(g, ocb) + f]
    wqk = np.empty((NG, 2 * NPAIR, 128, NCB, 128), np.float32)
    wv = np.empty((NG, 128, NCB, 512), np.float32)
    for g in range(NG):
        for ocb in range(2 * NPAIR):
            col0 = (0 if ocb < NPAIR else C) + g * 512 + (ocb % NPAIR) * 128
            blk = wqkvT[:, col0:col0 + 128].reshape(NCB, 128, 128)  # [cb, r, f]
            wqk[g, ocb] = blk.transpose(1, 0, 2)
        vblk = wqkvT[:, 2 * C + g * 512:2 * C + (g + 1) * 512].reshape(NCB, 128, 512)
        wv[g] = vblk.transpose(1, 0, 2)
    wo = np.empty((NCB, 128, NCB, 128), np.float32)
    for ocb in range(NCB):
        blk = woT[:, ocb * 128:(ocb + 1) * 128].reshape(NCB, 128, 128)
        wo[ocb] = blk.transpose(1, 0, 2)
    wfc = np.empty((NFB, 128, NCB, 128), np.float32)
    for fb in range(NFB):
        blk = wfcT[:, fb * 128:(fb + 1) * 128].reshape(NCB, 128, 128)
        wfc[fb] = blk.transpose(1, 0, 2)
    wproj = np.empty((NCB, 128, NFB, 128), np.float32)
    for ocb in range(NCB):
        blk = wprojT[:, ocb * 128:(ocb + 1) * 128].reshape(NFB, 128, 128)  # [fb, r, f]
        wproj[ocb] = blk.transpose(1, 0, 2)
    return {"wqk": wqk.astype(bf), "wv": wv.astype(bf), "wo": wo.astype(bf),
            "wfc": wfc.astype(bf), "wproj": wproj.astype(bf)}


def _prep(x, g1, w_qkv, w_o, g2, w_fc, w_proj):
    """Build the 8 per-core input maps (all host-side)."""
    x = np.asarray(x, np.float32)
    wmap = _prep_weights(g1, w_qkv, w_o, g2, w_fc, w_proj)
    bf = ml_dtypes.bfloat16

    in_maps = []
    for c in range(8):
        b, h = c // 2, c % 2
        # Fold-balanced query ownership: h=0 owns global 512-chunks {0,3},
        # h=1 owns {1,2}. Permute the sequence so the core's early (Y) chunk
        # sits at rotated [0,512) with its causal span inside rotated
        # [0,1024), and its late (X) chunk sits at rotated [1536,2048).
        order = [0, 1, 2, 3] if h == 0 else [1, 0, 3, 2]
        rot = np.concatenate([np.arange(ch * 512, (ch + 1) * 512) for ch in order])
        xT = np.ascontiguousarray(x[b][rot].T)
        gq_Y = rot[0:512]
        gq_X = rot[1536:2048]
        masks = np.zeros((NMV, 128, 512), np.float32)
        for m in range(8):
            kpos = rot[m * 128:(m + 1) * 128]
            masks[m] = (kpos[:, None] <= gq_Y[None, :])
        for m in range(8, 16):
            kpos = rot[m * 128:(m + 1) * 128]
            masks[m] = (kpos[:, None] <= gq_X[None, :])
        masks = np.ascontiguousarray(masks.transpose(1, 0, 2))  # [128, NMV, 512]
        in_maps.append({"xT": xT, "masks": masks.astype(bf), **wmap})
    return in_maps


def kernel(x, g1, w_qkv, w_o, g2, w_fc, w_proj, _trace=False, **_tk):
    from concourse.bass_utils import run_bass_kernel_spmd
    if "nc" not in _CACHE:
        _CACHE["nc"] = _build()
    nc = _CACHE["nc"]
    in_maps = _prep(x, g1, w_qkv, w_o, g2, w_fc, w_proj)
    res = run_bass_kernel_spmd(nc, in_maps, core_ids=list(range(8)),
                               trace=_trace, **_tk)
    _CACHE["last"] = res
    out = np.empty((B, T, C), np.float32)
    for c in range(8):
        b, h = c // 2, c % 2
        order = [0, 1, 2, 3] if h == 0 else [1, 0, 3, 2]
        o = res.results[c]["outT"]
        cY, cX = order[0], order[3]
        out[b, cY * 512:(cY + 1) * 512, :] = o[:, 0:512].T
        out[b, cX * 512:(cX + 1) * 512, :] = o[:, 512:1024].T
    return out


# revision 44
# speedup vs baseline: 1.2757x; 1.2757x over previous
"""Trainium2 Bass kernel for a GPT-style transformer block.

B=4, T=2048, C=1024, H=16 heads (D=64), FF=4096.
Sharding: 8 NeuronCores, core c = 2*b + h handles batch b and a causally
*folded* pair of 512-token query chunks (h=0: global chunks {0,3}, h=1:
{1,2}), so every core does the same balanced amount of causal attention
work. K/V are computed on-core over the full sequence. One uniform SPMD
program; per-core causality enters only through data (host-permuted x and
DMA'd multiplicative masks).

All matmuls run in bf16 (PSUM accumulates fp32) — on this hardware f32r
lowers to fp32_mode=HIGH at less than half bf16 throughput, and the
error budget (2e-2 relative) has orders of magnitude of headroom.
Activations stay channel-major ([channel, token]) end to end: LN
statistics are taken over the partition dim with ones-matmuls, softmax
runs without max subtraction (scores are bounded), and V is augmented
with a ones column so the softmax denominator falls out of the AV matmul
itself.

Phase schedule is software-pipelined for the Tensor engine's DVFS ramp
(full clock only under sustained load): group-1 QKV matmuls are emitted
as filler between group-0 attention visits, and phase A's LN chunks
interleave with group-0 QKV.
"""
import sys

sys.path.insert(0, "/opt/trn_rl_repo")

import numpy as np
import ml_dtypes
from contextlib import ExitStack

import concourse.bass as bass
import concourse.tile as tile
from concourse import bacc, mybir

F32 = mybir.dt.float32
BF16 = mybir.dt.bfloat16
AF = mybir.ActivationFunctionType
OP = mybir.AluOpType

B, T, C, H, D = 4, 2048, 1024, 16, 64
FF = 4 * C
TOK = T // 2          # tokens owned per core (two 512-chunks: Y and X)
NCB = C // 128        # 8 channel blocks
NFB = FF // 128       # 32 ff blocks
NG = 2                # head groups (asymmetric: 12 + 4 heads)
GPAIRS = (7, 1)       # pairs per group
GBASE = (0, 7)        # first global pair of each group
NSB = T // 128        # 16 key blocks
NMV = 16              # mask visits: 8 for Y (all) + 8 for X (last 8)

_CACHE = {}


def _build():
    nc = bacc.Bacc("TRN2", target_bir_lowering=False, debug=False, num_devices=8)

    xT_d = nc.dram_tensor("xT", [C, T], F32, kind="ExternalInput").ap()
    wqk_d = nc.dram_tensor("wqk", [H // 2, 2, 128, NCB, 128], BF16,
                           kind="ExternalInput").ap()
    wv_d = nc.dram_tensor("wv", [128, NCB, C], BF16, kind="ExternalInput").ap()
    wo_d = nc.dram_tensor("wo", [NCB, 128, NCB, 128], BF16, kind="ExternalInput").ap()
    wfc_d = nc.dram_tensor("wfc", [NFB, 128, NCB, 128], BF16, kind="ExternalInput").ap()
    wproj_d = nc.dram_tensor("wproj", [NCB, 128, NFB, 128], BF16,
                             kind="ExternalInput").ap()
    mask_d = nc.dram_tensor("masks", [128, NMV, 512], BF16, kind="ExternalInput").ap()
    out_d = nc.dram_tensor("outT", [C, TOK], F32, kind="ExternalOutput").ap()

    with tile.TileContext(nc) as tc, ExitStack() as top:
        persist = top.enter_context(tc.tile_pool(name="persist", bufs=1))
        ones_f = persist.tile([128, 128], F32)
        nc.vector.memset(ones_f[:], 1.0)
        ones_b = persist.tile([128, 128], BF16)
        nc.vector.tensor_copy(ones_b[:], ones_f[:])
        eps_t = persist.tile([128, 1], F32)
        nc.vector.memset(eps_t[:], 1e-5)
        y_sb = persist.tile([128, NCB, TOK], BF16)      # attention out (normalized)

        with tc.tile_pool(name="pLN", bufs=1) as pLN, \
             tc.tile_pool(name="pb", bufs=1) as pb, \
             tc.tile_pool(name="pbw", bufs=1) as pbw:
            lnr = pLN.tile([128, NCB, T], BF16)          # LN1(x), all tokens

            # ---------- Phase B: QKV for one group, as a fine generator ----
            def make_b(g, psB_):
                npair = GPAIRS[g]
                pbase = GBASE[g]
                nh = 2 * npair
                kT = pb.tile([128, npair, T], BF16, tag=f"kT{g}", name="kT")
                qT = pb.tile([128, npair, TOK], BF16, tag=f"qT{g}", name="qT")
                va = pb.tile([128, NSB, nh, 65], BF16, tag=f"vaug{g}", name="va")

                def gen():
                    wtk0 = pbw.tile([128, NCB, 128], BF16, tag="wk",
                                    name="wtk", bufs=2)
                    nc.sync.dma_start(wtk0[:], wqk_d[pbase, 1])
                    for sblk in range(NSB):
                        nc.vector.tensor_copy(
                            va[:, sblk, :, 64:65],
                            ones_f[:, 0:1].broadcast_to([128, nh, 1]))
                    yield
                    # V in head-halves (<=512 moving cols each), sblk-major
                    # so it pipelines with phase A
                    def v_batch(vw, h0, h1, s0, s1):
                        for sblk in range(s0, s1):
                            vps = psB_.tile([128, 64 * (h1 - h0)], F32,
                                            tag="vps", name="vps",
                                            padded_shape=[128, 512])
                            for cb in range(NCB):
                                nc.tensor.matmul(
                                    vps[:],
                                    lnr[:, cb, sblk * 128:(sblk + 1) * 128],
                                    vw[:, cb, :],
                                    start=(cb == 0), stop=(cb == NCB - 1))
                                if cb == 3:
                                    yield
                            nc.vector.tensor_copy(va[:, sblk, h0:h1, 0:64],
                                                  vps[:])
                            yield

                    def v_sweep():
                        hhs = ([(0, nh)] if nh <= 8 else
                               [(0, nh // 2), (nh // 2, nh)])
                        vws = []
                        for vi, (h0, h1) in enumerate(hhs):
                            vw = pbw.tile([128, NCB, 64 * (h1 - h0)], BF16,
                                          tag=f"vw{vi}", name="vw",
                                          padded_shape=[128, NCB, 512])
                            nc.sync.dma_start(
                                vw[:], wv_d[:, :, 64 * (2 * pbase + h0):
                                            64 * (2 * pbase + h1)])
                            vws.append((vw, h0, h1))
                        for s0 in range(0, NSB, 4):
                            for vw, h0, h1 in vws:
                                yield from v_batch(vw, h0, h1, s0, s0 + 4)

                    def k_sweep():
                        for p in range(npair):
                            if p == 0:
                                wtk = wtk0
                            else:
                                wtk = pbw.tile([128, NCB, 128], BF16,
                                               tag="wk", name="wtk", bufs=2)
                                nc.sync.dma_start(wtk[:], wqk_d[pbase + p, 1])
                            for n in range(4):
                                sl = slice(n * 512, (n + 1) * 512)
                                acc = psB_.tile([128, 512], F32, tag="qk",
                                                name="acc")
                                for cb in range(NCB):
                                    nc.tensor.matmul(acc[:], wtk[:, cb, :],
                                                     lnr[:, cb, sl],
                                                     start=(cb == 0),
                                                     stop=(cb == NCB - 1))
                                    if cb == 3:
                                        yield
                                nc.vector.tensor_copy(kT[:, p, sl], acc[:])
                                yield

                    yield from v_sweep()
                    yield from k_sweep()
                    # Q for the two owned 512-chunks (streamed weights)
                    for p in range(npair):
                        wtq = pbw.tile([128, NCB, 128], BF16, tag="wq",
                                       name="wtq", bufs=2)
                        nc.sync.dma_start(wtq[:], wqk_d[pbase + p, 0])
                        for ni, sl in enumerate((slice(0, 512),
                                                 slice(1536, 2048))):
                            acc = psB_.tile([128, 512], F32, tag="qk",
                                            name="acc")
                            for cb in range(NCB):
                                nc.tensor.matmul(acc[:], wtq[:, cb, :],
                                                 lnr[:, cb, sl],
                                                 start=(cb == 0),
                                                 stop=(cb == NCB - 1))
                                if cb == 3:
                                    yield
                            nc.vector.tensor_copy(
                                qT[:, p, ni * 512:(ni + 1) * 512], acc[:])
                            yield

                return kT, qT, va, gen()

            def drain(gen_, k=None):
                if gen_ is None:
                    return
                try:
                    if k is None:
                        for _ in gen_:
                            pass
                    else:
                        for _ in range(k):
                            next(gen_)
                except StopIteration:
                    pass

            psB0 = tc.alloc_tile_pool(name="psB0", bufs=3, space="PSUM")
            kT0, qT0, va0, bgen0 = make_b(0, psB0)

            # ---------- Phase A: LN1 per 512-token chunk, bf16 stats ----------
            with tc.tile_pool(name="pax", bufs=2) as pax, \
                 tc.tile_pool(name="paw", bufs=3) as paw, \
                 tc.tile_pool(name="psA", bufs=1, space="PSUM") as psA:

                def a_unit(tck):
                    sl = slice(tck * 512, (tck + 1) * 512)
                    sum_ps = psA.tile([1, 512], F32, tag="sum", name="sum_ps")
                    sq_ps = psA.tile([1, 512], F32, tag="sq", name="sq_ps")
                    xrs = []
                    for cb in range(NCB):
                        xf = paw.tile([128, 512], F32, tag=f"xf{cb % 2}",
                                      name="xf")
                        nc.sync.dma_start(xf[:], xT_d[cb * 128:(cb + 1) * 128, sl])
                        xr = pax.tile([128, 512], BF16, tag=f"xr{cb}", name="xr")
                        nc.scalar.copy(xr[:], xf[:])
                        x2 = paw.tile([128, 512], BF16, tag="x2", name="x2")
                        nc.vector.tensor_tensor(x2[:], xr[:], xr[:], OP.mult)
                        nc.tensor.matmul(sum_ps[:], ones_b[:, 0:1], xr[:],
                                         start=(cb == 0), stop=(cb == NCB - 1))
                        nc.tensor.matmul(sq_ps[:], ones_b[:, 0:1], x2[:],
                                         start=(cb == 0), stop=(cb == NCB - 1))
                        xrs.append(xr)
                    mean_r = par.tile([1, 512], F32, tag="mean", name="mean_r")
                    var_r = par.tile([1, 512], F32, tag="var", name="var_r")
                    nc.scalar.mul(mean_r[:], sum_ps[:], 1.0 / C)
                    nc.scalar.mul(var_r[:], sq_ps[:], 1.0 / C)
                    msq = par.tile([1, 512], F32, tag="msq", name="msq")
                    nc.vector.tensor_mul(msq[:], mean_r[:], mean_r[:])
                    nc.vector.tensor_sub(var_r[:], var_r[:], msq[:])
                    nc.scalar.activation(var_r[:], var_r[:], AF.Sqrt,
                                         bias=eps_t[0:1, :])
                    nc.vector.reciprocal_approx_fast(out=var_r[:], in_=var_r[:])
                    mb = paw.tile([128, 512], F32, tag="mb", name="mb")
                    rb = paw.tile([128, 512], F32, tag="rb", name="rb")
                    nc.gpsimd.partition_broadcast(mb[:], mean_r[:])
                    nc.gpsimd.partition_broadcast(rb[:], var_r[:])
                    mbf = pax.tile([128, 512], BF16, tag="mbf", name="mbf")
                    rbf = pax.tile([128, 512], BF16, tag="rbf", name="rbf")
                    nc.vector.tensor_copy(mbf[:], mb[:])
                    nc.vector.tensor_copy(rbf[:], rb[:])
                    for cb in range(NCB):
                        xc = paw.tile([128, 512], BF16, tag="xc", name="xc")
                        nc.vector.tensor_sub(xc[:], xrs[cb][:], mbf[:])
                        nc.vector.tensor_tensor(lnr[:, cb, sl], xc[:], rbf[:],
                                                OP.mult)

                # interleave A (4 units) with B0's V sweep (sblk-major)
                drain(bgen0, 1)           # ones columns + vw DMAs
                a_unit(0)
                drain(bgen0, 16)          # V lo+hi sblk 0..3
                a_unit(1)
                drain(bgen0, 16)          # V lo+hi sblk 4..7
                a_unit(2)
                drain(bgen0, 16)          # V lo+hi sblk 8..11
                a_unit(3)
                drain(bgen0)              # V rest + K + Q
            psB0.release()

            # ---------- Phase C ----------
            with tc.tile_pool(name="pcm", bufs=1) as pcm, \
                 tc.tile_pool(name="pct", bufs=2) as pct, \
                 tc.tile_pool(name="pcn", bufs=2) as pcn, \
                 tc.tile_pool(name="psS", bufs=2, space="PSUM") as psS, \
                 tc.tile_pool(name="psY", bufs=1, space="PSUM") as psY, \
                 tc.tile_pool(name="psB1", bufs=1, space="PSUM") as psB1:
                masks = pcm.tile([128, NMV, 512], BF16)
                nc.sync.dma_start(masks[:], mask_d)

                # deferred normalization: the odd-head halves collect in a
                # per-chunk batch buffer, DMA'd to y_sb[64:128] once per chunk
                # so the sync queue never waits on compute (keeps D prefetch
                # flowing)
                batch_state = {}

                def norm_tail(ysb, blk, qsl, ckey, npair_c, pbase_c):
                    den0 = pcn.tile([1, 1024], F32, tag="den0",
                                    name="den0", bufs=1)
                    nc.sync.dma_start(den0[:], ysb[64:65, :])
                    nc.vector.reciprocal_approx_fast(out=den0[:],
                                                     in_=den0[:])
                    rb01 = pcn.tile([64, 1024], F32, tag="rb01",
                                    name="rb01", bufs=1)
                    nc.gpsimd.partition_broadcast(rb01[:], den0[:],
                                                  channels=64)
                    if ckey not in batch_state:
                        yb = pcn.tile([64, 7, 512], BF16, tag="ybatch",
                                      name="yb", bufs=1)
                        batch_state[ckey] = yb
                    yb = batch_state[ckey]
                    nc.vector.tensor_tensor(y_sb[0:64, blk, qsl],
                                            ysb[0:64, 0:512],
                                            rb01[:, 0:512], OP.mult)
                    nc.vector.tensor_tensor(yb[:, blk - pbase_c, :],
                                            ysb[0:64, 512:1024],
                                            rb01[:, 512:1024], OP.mult)
                    if blk - pbase_c == npair_c - 1:
                        nc.sync.dma_start(
                            y_sb[64:128, pbase_c:pbase_c + npair_c, qsl],
                            yb[:, 0:npair_c, :])
                        del batch_state[ckey]

                def c_phase(g, kT, qT, va, filler, pending):
                    npair = GPAIRS[g]
                    pbase = GBASE[g]

                    # zero-padded moving q for full-tile matmuls: the dead
                    # half contributes 0 to the K=128 contraction. Prepared
                    # one unit ahead so unit starts never wait on DVE.
                    units = [(c, p) for c in (1, 0) for p in range(npair)]

                    def prep_q(c, p):
                        qsl_p = slice(c * 512, (c + 1) * 512)
                        qe_u = pct.tile([128, 512], BF16, tag="qe",
                                        name="qe_u", bufs=3)
                        nc.vector.memset(qe_u[64:128, :], 0.0)
                        nc.vector.tensor_copy(qe_u[0:64, :],
                                              qT[0:64, p, qsl_p])
                        qo_u = pct.tile([128, 512], BF16, tag="qo",
                                        name="qo_u", bufs=3)
                        nc.vector.memset(qo_u[0:64, :], 0.0)
                        nc.vector.tensor_copy(qo_u[64:128, :],
                                              qT[64:128, p, qsl_p])
                        return qe_u, qo_u

                    prepped = prep_q(*units[0])
                    for ui, (chunk, pair) in enumerate(units):
                        trip = 8 if chunk == 0 else 16
                        mask_lo = 0 if chunk == 0 else 8
                        qsl = slice(chunk * 512, (chunk + 1) * 512)
                        if True:
                            qe_u, qo_u = prepped
                            y01 = psY.tile([65, 1024], F32, tag="y01",
                                           name="y01")

                            def av(j, p01r):
                                st = (j == 0)
                                sp = (j == trip - 1)
                                nc.tensor.matmul(y01[:, 0:512],
                                                 va[:, j, 2 * pair, :],
                                                 p01r[:, 0:512], start=st,
                                                 stop=sp)
                                nc.tensor.matmul(y01[:, 512:1024],
                                                 va[:, j, 2 * pair + 1, :],
                                                 p01r[:, 512:1024], start=st,
                                                 stop=sp)

                            # software pipeline: AV lags the score/exp
                            # stream by TWO visits so the PE never waits on
                            # the exp->mask chain and Act never waits on PE
                            pend = []
                            for j in range(trip):
                                jsl = slice(j * 128, (j + 1) * 128)
                                s01 = psS.tile([128, 1024], F32, tag="s01",
                                               name="s01")
                                nc.tensor.matmul(s01[:, 0:512],
                                                 kT[:, pair, jsl],
                                                 qe_u[:], start=True,
                                                 stop=True)
                                nc.tensor.matmul(s01[:, 512:1024],
                                                 kT[:, pair, jsl],
                                                 qo_u[:], start=True,
                                                 stop=True)
                                p01r = pct.tile([128, 1024], BF16, tag="p01r",
                                                name="p01r", bufs=4)
                                nc.scalar.activation(p01r[:], s01[:],
                                                     AF.Exp, scale=0.125)
                                if j >= mask_lo:
                                    m2 = masks[:, j:j + 1, :].broadcast_to(
                                        [128, 2, 512])
                                    nc.vector.tensor_tensor(p01r[:], p01r[:],
                                                            m2, OP.mult)
                                pend.append((j, p01r))
                                if len(pend) > 2:
                                    av(*pend.pop(0))
                                if j == 1 and ui + 1 < len(units):
                                    prepped = prep_q(*units[ui + 1])
                                if j % 2 == 0:
                                    drain(filler, 1)
                            for u in pend:
                                av(*u)
                            # evacuate PSUM now; defer the denom/mult tail
                            # one unit so its DMAs never stall the sync queue
                            ysb = pcn.tile([65, 1024], F32, tag="ysb",
                                           name="ysb")
                            nc.vector.tensor_copy(ysb[:, 0:512],
                                                  y01[:, 0:512])
                            nc.vector.tensor_copy(ysb[:, 512:1024],
                                                  y01[:, 512:1024])
                            if pending[0] is not None:
                                norm_tail(*pending[0])
                            pending[0] = (ysb, pbase + pair, qsl,
                                          (g, chunk), npair, pbase)

                kT1, qT1, va1, bgen1 = make_b(1, psB1)
                drain(bgen1, 1)
                pending = [None]
                c_phase(0, kT0, qT0, va0, bgen1, pending)
                drain(bgen1)
                c_phase(1, kT1, qT1, va1, None, pending)
                if pending[0] is not None:
                    norm_tail(*pending[0])

        # ============ Phase D: out-projection + residual + LN2 ============
        with tc.tile_pool(name="pd", bufs=1) as pd, \
             tc.tile_pool(name="pew", bufs=3) as pew:
            x1 = pd.tile([128, NCB, TOK], F32)           # x + attn
            ln2r = pd.tile([128, NCB, TOK], BF16)
            wfc_pre = []
            for fb in range(3):
                wt = pew.tile([128, NCB, 128], BF16, tag="fwt", name="wt")
                nc.sync.dma_start(wt[:], wfc_d[fb])
                wfc_pre.append(wt)
            with tc.tile_pool(name="pdw", bufs=2) as pdw, \
                 tc.tile_pool(name="pdt", bufs=2) as pdt, \
                 tc.tile_pool(name="pxr", bufs=1) as pxr, \
                 tc.tile_pool(name="psD", bufs=2, space="PSUM") as psD, \
                 tc.tile_pool(name="psE", bufs=1, space="PSUM") as psE:
                sum_c = [psE.tile([1, 512], F32, tag=f"sum{ch}", name="sum_c")
                         for ch in range(2)]
                sq_c = [psE.tile([1, 512], F32, tag=f"sq{ch}", name="sq_c")
                        for ch in range(2)]
                xr2s = []
                for ocb in range(NCB):
                    wt = pdw.tile([128, NCB, 128], BF16, tag="wt")
                    nc.sync.dma_start(wt[:], wo_d[ocb])
                    acc = psD.tile([128, 1024], F32, tag="proj")
                    for cblk in range(NCB):
                        for n2 in range(2):
                            nc.tensor.matmul(acc[:, n2 * 512:(n2 + 1) * 512],
                                             wt[:, cblk, :],
                                             y_sb[:, cblk, n2 * 512:(n2 + 1) * 512],
                                             start=(cblk == 0), stop=(cblk == NCB - 1))
                    xf = pdt.tile([128, TOK], F32, tag="xres")
                    # own tokens: rotated cols [0:512) (Y) and [1536:2048) (X)
                    nc.sync.dma_start(xf[:, 0:512],
                                      xT_d[ocb * 128:(ocb + 1) * 128, 0:512])
                    nc.sync.dma_start(xf[:, 512:1024],
                                      xT_d[ocb * 128:(ocb + 1) * 128, 1536:2048])
                    nc.vector.tensor_add(x1[:, ocb, :], acc[:], xf[:])
                    # LN2 inputs (stat matmuls emitted after all proj chains
                    # so they don't stall the PE queue on DVE/Act)
                    xr2 = pxr.tile([128, TOK], BF16, tag=f"xr2_{ocb}",
                                   name="xr2")
                    nc.scalar.copy(xr2[:], x1[:, ocb, :])
                    sq2 = pxr.tile([128, TOK], BF16, tag=f"sq2_{ocb}",
                                   name="sq2")
                    nc.vector.tensor_tensor(sq2[:], xr2[:], xr2[:], OP.mult)
                    xr2s.append((xr2, sq2))
                    if ocb > 0:
                        pxr2, psq2 = xr2s[ocb - 1]
                        for ch in range(2):
                            csl = slice(ch * 512, (ch + 1) * 512)
                            nc.tensor.matmul(sum_c[ch][:], ones_b[:, 0:1],
                                             pxr2[:, csl],
                                             start=(ocb == 1), stop=False)
                            nc.tensor.matmul(sq_c[ch][:], ones_b[:, 0:1],
                                             psq2[:, csl],
                                             start=(ocb == 1), stop=False)
                xr2, sq2 = xr2s[NCB - 1]
                for ch in range(2):
                    csl = slice(ch * 512, (ch + 1) * 512)
                    nc.tensor.matmul(sum_c[ch][:], ones_b[:, 0:1],
                                     xr2[:, csl], start=False, stop=True)
                    nc.tensor.matmul(sq_c[ch][:], ones_b[:, 0:1],
                                     sq2[:, csl], start=False, stop=True)
                # LN2 normalize (full-width ops)
                mb = pdt.tile([128, TOK], F32, tag="mb", name="mb", bufs=1)
                rb = pdt.tile([128, TOK], F32, tag="rb", name="rb", bufs=1)
                mbf = pdt.tile([128, TOK], BF16, tag="mbf", name="mbf", bufs=1)
                rbf = pdt.tile([128, TOK], BF16, tag="rbf", name="rbf", bufs=1)
                for ch in range(2):
                    csl = slice(ch * 512, (ch + 1) * 512)
                    mean_r = pdt.tile([1, 512], F32, tag="mean", name="mean_r")
                    var_r = pdt.tile([1, 512], F32, tag="var", name="var_r")
                    nc.scalar.mul(mean_r[:], sum_c[ch][:], 1.0 / C)
                    nc.scalar.mul(var_r[:], sq_c[ch][:], 1.0 / C)
                    msq = pdt.tile([1, 512], F32, tag="msq", name="msq")
                    nc.vector.tensor_mul(msq[:], mean_r[:], mean_r[:])
                    nc.vector.tensor_sub(var_r[:], var_r[:], msq[:])
                    nc.scalar.activation(var_r[:], var_r[:], AF.Sqrt,
                                         bias=eps_t[0:1, :])
                    nc.vector.reciprocal_approx_fast(out=var_r[:], in_=var_r[:])
                    nc.gpsimd.partition_broadcast(mb[:, csl], mean_r[:])
                    nc.gpsimd.partition_broadcast(rb[:, csl], var_r[:])
                nc.vector.tensor_copy(mbf[:], mb[:])
                nc.vector.tensor_copy(rbf[:], rb[:])
                for cb in range(NCB):
                    xc = pdt.tile([128, TOK], BF16, tag="xc", name="xc")
                    nc.vector.tensor_sub(xc[:], xr2s[cb][0][:], mbf[:])
                    nc.vector.tensor_tensor(ln2r[:, cb, :], xc[:], rbf[:],
                                            OP.mult)

            # ============ Phase E: MLP (single pass over all own tokens) ====
            with tc.tile_pool(name="ph", bufs=1) as ph, \
                 tc.tile_pool(name="pepw", bufs=2) as pepw, \
                 tc.tile_pool(name="peo", bufs=3) as peo, \
                 tc.tile_pool(name="psF", bufs=2, space="PSUM") as psF:
                h_r = ph.tile([128, NFB, TOK], BF16)
                for fb in range(NFB):
                    if fb < 3:
                        wt = wfc_pre[fb]
                    else:
                        wt = pew.tile([128, NCB, 128], BF16, tag="fwt",
                                      name="wt")
                        nc.sync.dma_start(wt[:], wfc_d[fb])
                    fc = psF.tile([128, 1024], F32, tag="fc")
                    for cb in range(NCB):
                        for n2 in range(2):
                            nc.tensor.matmul(fc[:, n2 * 512:(n2 + 1) * 512],
                                             wt[:, cb, :],
                                             ln2r[:, cb, n2 * 512:(n2 + 1) * 512],
                                             start=(cb == 0),
                                             stop=(cb == NCB - 1))
                    nc.scalar.activation(h_r[:, fb, :], fc[:], AF.Gelu)
                for ocb in range(NCB):
                    wt = pepw.tile([128, NFB, 128], BF16, tag="pwt")
                    nc.sync.dma_start(wt[:], wproj_d[ocb])
                    acc = psF.tile([128, 1024], F32, tag="pacc")
                    for fb in range(NFB):
                        for n2 in range(2):
                            nc.tensor.matmul(acc[:, n2 * 512:(n2 + 1) * 512],
                                             wt[:, fb, :],
                                             h_r[:, fb, n2 * 512:(n2 + 1) * 512],
                                             start=(fb == 0), stop=(fb == NFB - 1))
                    of = peo.tile([128, TOK], F32, tag="of")
                    nc.vector.tensor_add(of[:], acc[:], x1[:, ocb, :])
                    nc.sync.dma_start(out_d[ocb * 128:(ocb + 1) * 128, :], of[:])

    nc.compile()
    return nc


def _prep_weights(g1, w_qkv, w_o, g2, w_fc, w_proj):
    g1 = np.asarray(g1, np.float32)
    g2 = np.asarray(g2, np.float32)
    wqkvT = np.ascontiguousarray((np.asarray(w_qkv, np.float32) * g1[None, :]).T)
    woT = np.ascontiguousarray(np.asarray(w_o, np.float32).T)
    wfcT = np.ascontiguousarray((np.asarray(w_fc, np.float32) * g2[None, :]).T)
    wprojT = np.ascontiguousarray(np.asarray(w_proj, np.float32).T)

    bf = ml_dtypes.bfloat16
    # wqk[p, s, r, cb, f] = wqkvT[cb*128 + r, s*C + p*128 + f]  (s: 0=q, 1=k)
    wqk = np.empty((H // 2, 2, 128, NCB, 128), np.float32)
    for p in range(H // 2):
        for s in range(2):
            col0 = s * C + p * 128
            blk = wqkvT[:, col0:col0 + 128].reshape(NCB, 128, 128)  # [cb, r, f]
            wqk[p, s] = blk.transpose(1, 0, 2)
    vblk = wqkvT[:, 2 * C:3 * C].reshape(NCB, 128, C)
    wv = np.ascontiguousarray(vblk.transpose(1, 0, 2))  # [128, NCB, C]
    wo = np.empty((NCB, 128, NCB, 128), np.float32)
    for ocb in range(NCB):
        blk = woT[:, ocb * 128:(ocb + 1) * 128].reshape(NCB, 128, 128)
        wo[ocb] = blk.transpose(1, 0, 2)
    wfc = np.empty((NFB, 128, NCB, 128), np.float32)
    for fb in range(NFB):
        blk = wfcT[:, fb * 128:(fb + 1) * 128].reshape(NCB, 128, 128)
        wfc[fb] = blk.transpose(1, 0, 2)
    wproj = np.empty((NCB, 128, NFB, 128), np.float32)
    for ocb in range(NCB):
        blk = wprojT[:, ocb * 128:(ocb + 1) * 128].reshape(NFB, 128, 128)  # [fb, r, f]
        wproj[ocb] = blk.transpose(1, 0, 2)
    return {"wqk": wqk.astype(bf), "wv": wv.astype(bf), "wo": wo.astype(bf),
            "wfc": wfc.astype(bf), "wproj": wproj.astype(bf)}


def _prep(x, g1, w_qkv, w_o, g2, w_fc, w_proj):
    """Build the 8 per-core input maps (all host-side)."""
    x = np.asarray(x, np.float32)
    wmap = _prep_weights(g1, w_qkv, w_o, g2, w_fc, w_proj)
    bf = ml_dtypes.bfloat16

    in_maps = []
    for c in range(8):
        b, h = c // 2, c % 2
        # Fold-balanced query ownership: h=0 owns global 512-chunks {0,3},
        # h=1 owns {1,2}. Permute the sequence so the core's early (Y) chunk
        # sits at rotated [0,512) with its causal span inside rotated
        # [0,1024), and its late (X) chunk sits at rotated [1536,2048).
        order = [0, 1, 2, 3] if h == 0 else [1, 0, 3, 2]
        rot = np.concatenate([np.arange(ch * 512, (ch + 1) * 512) for ch in order])
        xT = np.ascontiguousarray(x[b][rot].T)
        gq_Y = rot[0:512]
        gq_X = rot[1536:2048]
        masks = np.zeros((NMV, 128, 512), np.float32)
        for m in range(8):
            kpos = rot[m * 128:(m + 1) * 128]
            masks[m] = (kpos[:, None] <= gq_Y[None, :])
        for m in range(8, 16):
            kpos = rot[m * 128:(m + 1) * 128]
            masks[m] = (kpos[:, None] <= gq_X[None, :])
        masks = np.ascontiguousarray(masks.transpose(1, 0, 2))  # [128, NMV, 512]
        in_maps.append({"xT": xT, "masks": masks.astype(bf), **wmap})
    return in_maps


def kernel(x, g1, w_qkv, w_o, g2, w_fc, w_proj, _trace=False, **_tk):
    from concourse.bass_utils import run_bass_kernel_spmd
    if "nc" not in _CACHE:
        _CACHE["nc"] = _build()
    nc = _CACHE["nc"]
    in_maps = _prep(x, g1, w_qkv, w_o, g2, w_fc, w_proj)
    res = run_bass_kernel_spmd(nc, in_maps, core_ids=list(range(8)),
                               trace=_trace, **_tk)
    _CACHE["last"] = res
    out = np.empty((B, T, C), np.float32)
    for c in range(8):
        b, h = c // 2, c % 2
        order = [0, 1, 2, 3] if h == 0 else [1, 0, 3, 2]
        o = res.results[c]["outT"]
        cY, cX = order[0], order[3]
        out[b, cY * 512:(cY + 1) * 512, :] = o[:, 0:512].T
        out[b, cX * 512:(cX + 1) * 512, :] = o[:, 512:1024].T
    return out
